# revision 2
# baseline (speedup 1.0000x reference)
"""DeepseekV2 MLA attention (B=1, S=2048, H=4096, NH=32) on 8 TRN2 cores.

Sharding: tensor-parallel over heads (4 heads/core).  The q_a projection +
RMSNorm runs data-parallel over sequence (each core does its 256-row slice)
and is AllGathered; the (cheaper) kv_a front is replicated per core so the
K/V projections can proceed while the AllGather is in flight.  Each core
emits a partial output projection (its head slice of Wo); the host sums the
8 partials.

All matmul operands are pre-transposed/packed on the HOST into T-layout
([feature, seq]) so the PE always contracts over the partition dim with zero
on-device transposes.  RMSNorm ln weights and the softmax scale are folded
into Wqb/Wkvb host-side.  Attention runs as logits^T [k, q]: softmax over
the partition axis via ones-matmul denominators, no max subtraction (logits
are O(5) for randn inputs), mask applied as data (causal tiles skipped only
when the host verifies the mask is exactly causal).

Matmuls run in float32r (full-rate PE; ~3e-4 rel err end to end).
"""

import ctypes
import os
import numpy as np

import concourse.bass as bass
import concourse.mybir as mybir
from concourse.tile import TileContext
import concourse.bass_utils as bass_utils
from concourse.bass_utils import run_bass_kernel_spmd

bass_utils.upload_artifacts = lambda tmpdir: tmpdir  # no artifact bucket here

S = 2048
H = 4096
NCORES = 8
NHC = 4            # heads per core
NOPE, ROPE, VD = 128, 64, 128
QHD = NOPE + ROPE  # 192
QLR, KVLR = 1536, 512
BASE = 10000.0
EPS = 1e-6
SCALE = QHD ** -0.5
P = 128
SC = 512           # seq chunk (local phases)
SLC = S // NCORES  # 256, per-core front slice
NSC = S // SC      # 4
NKB = S // P       # 16 key blocks
FR = mybir.dt.float32r
F32 = mybir.dt.float32
AF = mybir.ActivationFunctionType

N_KI = H // P      # 32 front contraction tiles
NQB = QLR // P     # 12
NKVB = KVLR // P   # 4
FB_W = [P] * NQB + [P] * NKVB + [ROPE]  # 17 front output blocks
N_FB = len(FB_W)


def axon_reset():
    import jax
    jax.devices()
    lib = ctypes.CDLL('/opt/axon/libaxon_pjrt.so')
    lib.axon_reset.restype = ctypes.c_int64
    return lib.axon_reset()


def split_multiwaits(nc, cap=1):
    """This walrus pin allows only `cap` sync-waits per instruction; spill
    extras onto same-engine NoOps inserted just before the instruction."""
    for f in nc.m.functions:
        for b in f.blocks:
            li = b.instructions
            out = []
            changed = False
            for inst in list(li):
                si = getattr(inst, "sync_info", None)
                waits = list(si.on_wait) if si is not None and si.on_wait else []
                if len(waits) > cap:
                    changed = True
                    extra, keep = waits[:-cap], waits[-cap:]
                    for j in range(0, len(extra), cap):
                        out.append(mybir.InstNoOp(
                            name=nc.get_next_instruction_name(),
                            engine=inst.engine, ins=[], outs=[],
                            sync_info=mybir.SyncInfo(
                                on_wait=extra[j:j + cap], on_update=[]),
                            bass_nofuse=True,
                        ))
                    inst.sync_info = mybir.SyncInfo(
                        on_wait=keep, on_update=list(si.on_update))
                out.append(inst)
            if changed:
                li[:] = out


def build(causal: bool) -> bass.Bass:
    nc = bass.Bass()
    hT = nc.declare_dram_parameter("hT", [H, S], F32, isOutput=False)
    hTs = nc.declare_dram_parameter("hTs", [H, SLC], F32, isOutput=False)
    maskT = nc.declare_dram_parameter("maskT", [S, S], F32, isOutput=False)
    Wp = nc.declare_dram_parameter("Wp", [P, N_FB * N_KI * P], F32, isOutput=False)
    Wqb_p = nc.declare_dram_parameter("Wqb_p", [P, NQB * NHC * QHD], F32, isOutput=False)
    Wkvb_p = nc.declare_dram_parameter("Wkvb_p", [P, NKVB * NHC * (NOPE + VD)], F32, isOutput=False)
    Wo_p = nc.declare_dram_parameter("Wo_p", [P, NKVB * H], F32, isOutput=False)
    cq = nc.declare_dram_parameter("cq", [ROPE, S], F32, isOutput=False)
    sq = nc.declare_dram_parameter("sq", [ROPE, S], F32, isOutput=False)
    outT = nc.declare_dram_parameter("outT", [H, S], F32, isOutput=True)

    Wp3 = Wp.rearrange("p (fk w) -> p fk w", w=P)        # [P, 17*32, 128]
    Wqb3 = Wqb_p.rearrange("p (k w) -> p k w", k=NQB)    # [P, 12, 768]
    Wkvb3 = Wkvb_p.rearrange("p (k w) -> p k w", k=NKVB)  # [P, 4, 1024]
    Wo3 = Wo_p.rearrange("p (k w) -> p k w", k=NKVB)     # [P, 4, 4096]

    def fr(ap):
        return ap.bitcast(FR)

    with TileContext(nc) as tc:
        with (
            tc.tile_pool(name="dram", bufs=1, space="DRAM") as dpool,
            tc.tile_pool(name="const", bufs=1) as cpool,
        ):
            kvnT = dpool.tile([KVLR, S], F32)
            qnT = dpool.tile([NHC * NOPE, S], F32)
            qrT = dpool.tile([NHC * ROPE, S], F32)
            kpeT = dpool.tile([ROPE, S], F32)
            onT = dpool.tile([NHC * VD, S], F32)
            cc_q_in = dpool.tile([QLR, SLC], F32)
            cc_q_out = dpool.tile([NCORES, QLR, SLC], F32, addr_space="Shared")
            ones_f = cpool.tile([P, 1], F32)
            nc.vector.memset(ones_f[:], 1.0)
            ones_rf = cpool.tile([1, P], F32)
            nc.vector.memset(ones_rf[:], 1.0)
            ones_t = cpool.tile([P, 1], FR)
            nc.scalar.copy(ones_t[:], ones_f[:])
            ones_row = cpool.tile([1, P], FR)
            nc.scalar.copy(ones_row[:], ones_rf[:])

            # ------------- Phase 1: front projections + RMSNorm + k rope
            with (
                tc.tile_pool(name="hcol", bufs=1) as hpool,
                tc.tile_pool(name="wfr", bufs=2) as wpool,
                tc.tile_pool(name="raw", bufs=1) as rpool,
                tc.tile_pool(name="nrm", bufs=2) as npool,
                tc.tile_pool(name="ckr", bufs=1) as ckpool,
                tc.tile_pool(name="ps", bufs=3, space="PSUM") as pspool,
                tc.tile_pool(name="ps1", bufs=1, space="PSUM") as ps1pool,
            ):
                # --- 1q: q_a on the local 256-col slice, then AllGather
                hqs = []
                for ki in range(N_KI):
                    ht = hpool.tile([P, SLC], FR, tag=f"h{ki}", name=f"hq{ki}")
                    nc.gpsimd.dma_start(out=ht[:], in_=hTs[ki * P:(ki + 1) * P, :])
                    hqs.append(ht)
                qraws = []
                sq_qp = ps1pool.tile([1, SLC], F32, tag="sq_q")
                for fb in range(NQB):
                    wt = wpool.tile([P, N_KI, P], FR, tag="w", name=f"wq{fb}")
                    nc.gpsimd.dma_start(
                        out=wt[:], in_=Wp3[:, fb * N_KI:(fb + 1) * N_KI, :])
                    ps = pspool.tile([P, SLC], F32, tag="ps", name=f"psq{fb}")
                    for ki in range(N_KI):
                        nc.tensor.matmul(ps[:], lhsT=fr(wt[:, ki, :]), rhs=hqs[ki][:],
                                         start=(ki == 0), stop=(ki == N_KI - 1))
                    raw = rpool.tile([P, SLC], F32, tag=f"r{fb}", name=f"rq{fb}")
                    nc.scalar.copy(raw[:], ps[:])
                    qraws.append(raw)
                    sqt = npool.tile([P, SLC], FR, tag="sqt", name=f"sqtq{fb}")
                    nc.vector.tensor_mul(sqt[:], raw[:], raw[:])
                    nc.tensor.matmul(sq_qp[:], lhsT=ones_t[:], rhs=sqt[:],
                                     start=(fb == 0), stop=(fb == NQB - 1))
                ms = npool.tile([1, SLC], F32, tag="ms", name="msq")
                nc.scalar.activation(ms[:], sq_qp[:], AF.Copy, scale=1.0 / QLR, bias=EPS)
                rc = npool.tile([1, SLC], F32, tag="rc", name="rcq")
                nc.vector.reciprocal(rc[:], ms[:])
                rs = npool.tile([1, SLC], FR, tag="rs", name="rsq")
                nc.scalar.activation(rs[:], rc[:], AF.Sqrt)
                bps = ps1pool.tile([P, SLC], F32, tag="bps", name="bpsq")
                nc.tensor.matmul(bps[:], lhsT=ones_row[:], rhs=rs[:], start=True, stop=True)
                rb = npool.tile([P, SLC], F32, tag="rb", name="rbq")
                nc.scalar.copy(rb[:], bps[:])
                for j in range(NQB):
                    nt = npool.tile([P, SLC], F32, tag="nt", name=f"ntq{j}")
                    nc.vector.tensor_mul(nt[:], qraws[j][:], rb[:])
                    nc.gpsimd.dma_start(out=cc_q_in[j * P:(j + 1) * P, :], in_=nt[:])
                nc.gpsimd.collective_compute(
                    "AllGather", mybir.AluOpType.bypass,
                    replica_groups=[list(range(NCORES))],
                    ins=[cc_q_in.opt()], outs=[cc_q_out.opt()])

                # --- 1kv: kv_a + rope over the full sequence (replicated)
                ck_t = ckpool.tile([ROPE, S], F32, tag="ck")
                sk_t = ckpool.tile([ROPE, S], F32, tag="sk")
                nc.gpsimd.dma_start(out=ck_t[:], in_=cq[:, :])
                nc.gpsimd.dma_start(out=sk_t[:], in_=sq[:, :])
                for sc in range(NSC):
                    ssl = slice(sc * SC, (sc + 1) * SC)
                    hts = []
                    for ki in range(N_KI):
                        ht = hpool.tile([P, SC], FR, tag=f"h{ki}", name=f"hk{ki}_{sc}")
                        nc.gpsimd.dma_start(out=ht[:], in_=hT[ki * P:(ki + 1) * P, ssl])
                        hts.append(ht)
                    raws = []
                    sq_kv = ps1pool.tile([1, SC], F32, tag="sq_kv")
                    for fbi, fb in enumerate(range(NQB, N_FB)):
                        w = FB_W[fb]
                        wt = wpool.tile([P, N_KI, P], FR, tag="w", name=f"wk{fb}_{sc}")
                        nc.gpsimd.dma_start(
                            out=wt[:], in_=Wp3[:, fb * N_KI:(fb + 1) * N_KI, :])
                        ps = pspool.tile([P, SC], F32, tag="ps", name=f"psk{fb}_{sc}")
                        for ki in range(N_KI):
                            nc.tensor.matmul(ps[:w, :], lhsT=fr(wt[:, ki, :w]), rhs=hts[ki][:],
                                             start=(ki == 0), stop=(ki == N_KI - 1))
                        raw = rpool.tile([P, SC], F32, tag=f"r{fb}", name=f"rk{fb}_{sc}")
                        nc.scalar.copy(raw[:w, :], ps[:w, :])
                        raws.append(raw)
                        if fb < NQB + NKVB:
                            sqt = npool.tile([P, SC], FR, tag="sqt", name=f"sqtk{fb}_{sc}")
                            nc.vector.tensor_mul(sqt[:], raw[:], raw[:])
                            nc.tensor.matmul(sq_kv[:], lhsT=ones_t[:], rhs=sqt[:],
                                             start=(fb == NQB), stop=(fb == NQB + NKVB - 1))
                    ms = npool.tile([1, SC], F32, tag="ms", name=f"msk{sc}")
                    nc.scalar.activation(ms[:], sq_kv[:], AF.Copy, scale=1.0 / KVLR, bias=EPS)
                    rc = npool.tile([1, SC], F32, tag="rc", name=f"rck{sc}")
                    nc.vector.reciprocal(rc[:], ms[:])
                    rs = npool.tile([1, SC], FR, tag="rs", name=f"rsk{sc}")
                    nc.scalar.activation(rs[:], rc[:], AF.Sqrt)
                    bps = ps1pool.tile([P, SC], F32, tag="bps", name=f"bpsk{sc}")
                    nc.tensor.matmul(bps[:], lhsT=ones_row[:], rhs=rs[:], start=True, stop=True)
                    rb = npool.tile([P, SC], F32, tag="rb", name=f"rbk{sc}")
                    nc.scalar.copy(rb[:], bps[:])
                    for j in range(NKVB):
                        nt = npool.tile([P, SC], F32, tag="nt", name=f"ntk{j}_{sc}")
                        nc.vector.tensor_mul(nt[:], raws[j][:], rb[:])
                        nc.gpsimd.dma_start(out=kvnT[j * P:(j + 1) * P, ssl], in_=nt[:])
                    kraw = raws[NKVB]
                    ksw = npool.tile([ROPE, SC], F32, tag="ksw", name=f"ksw{sc}")
                    nc.gpsimd.dma_start(out=ksw[0:32, :], in_=kraw[32:64, :])
                    nc.gpsimd.dma_start(out=ksw[32:64, :], in_=kraw[0:32, :])
                    ka = npool.tile([ROPE, SC], F32, tag="ka", name=f"ka{sc}")
                    nc.vector.tensor_mul(ka[:], kraw[:ROPE, :], ck_t[:, ssl])
                    kb_ = npool.tile([ROPE, SC], F32, tag="kb", name=f"kb{sc}")
                    nc.vector.tensor_mul(kb_[:], ksw[:], sk_t[:, ssl])
                    ko = npool.tile([ROPE, SC], F32, tag="ko", name=f"ko{sc}")
                    nc.vector.tensor_add(ko[:], ka[:], kb_[:])
                    nc.gpsimd.dma_start(out=kpeT[:, ssl], in_=ko[:])

            if True:
                # ------------- Phase 2a-kv: K_nope / V projections (local data,
                # runs while the q AllGather is in flight)
                kv2pool = tc.tile_pool(name="kv2", bufs=1)
                kv2 = kv2pool.__enter__()
                KN = [kv2.tile([NOPE, S], FR, tag=f"kn{h}", name=f"kn{h}") for h in range(NHC)]
                V = [kv2.tile([P, NHC, VD], FR, tag=f"v{sb}", name=f"v{sb}") for sb in range(NKB)]
                kpe_sb = kv2.tile([ROPE, S], FR, tag="kpe")
                nc.gpsimd.dma_start(out=kpe_sb[:], in_=kpeT[:, :])
                with (
                    tc.tile_pool(name="whk", bufs=1) as whpool,
                    tc.tile_pool(name="acol2", bufs=1) as apool,
                    tc.tile_pool(name="ps2k", bufs=2, space="PSUM") as ps2pool,
                ):
                    wkvb_t = whpool.tile([P, NKVB, NHC * (NOPE + VD)], FR, tag="wkvb")
                    nc.gpsimd.dma_start(out=wkvb_t[:], in_=Wkvb3[:, :, :])
                    for sc in range(NSC):
                        ssl = slice(sc * SC, (sc + 1) * SC)
                        kvc = []
                        for j in range(NKVB):
                            t = apool.tile([P, SC], FR, tag=f"kv{j}", name=f"kvc{j}_{sc}")
                            nc.gpsimd.dma_start(out=t[:], in_=kvnT[j * P:(j + 1) * P, ssl])
                            kvc.append(t)
                        for h in range(NHC):
                            koff = h * (NOPE + VD)
                            ps = ps2pool.tile([P, SC], F32, tag="p2", name=f"p2k{h}_{sc}")
                            for j in range(NKVB):
                                nc.tensor.matmul(ps[:], lhsT=fr(wkvb_t[:, j, koff:koff + NOPE]),
                                                 rhs=kvc[j][:],
                                                 start=(j == 0), stop=(j == NKVB - 1))
                            nc.scalar.copy(KN[h][:, ssl], ps[:])
                            for sb in range(SC // P):
                                psv = ps2pool.tile([P, VD], F32, tag="pv", name=f"pv{h}_{sc}_{sb}")
                                for j in range(NKVB):
                                    nc.tensor.matmul(
                                        psv[:], lhsT=fr(kvc[j][:, sb * P:(sb + 1) * P]),
                                        rhs=fr(wkvb_t[:, j, koff + NOPE:koff + NOPE + VD]),
                                        start=(j == 0), stop=(j == NKVB - 1))
                                nc.scalar.copy(V[sc * (SC // P) + sb][:, h, :], psv[:])

                # ------------- Phase 2a-q: Q projections + rope (consumes the
                # AllGathered q_a_n, rank-chunked)
                with (
                    tc.tile_pool(name="whq", bufs=1) as whpool,
                    tc.tile_pool(name="acol", bufs=1) as apool,
                    tc.tile_pool(name="rope", bufs=2) as ropepool,
                    tc.tile_pool(name="ps2", bufs=2, space="PSUM") as ps2pool,
                ):
                    wqb_t = whpool.tile([P, NQB, NHC * QHD], FR, tag="wqb")
                    nc.gpsimd.dma_start(out=wqb_t[:], in_=Wqb3[:, :, :])
                    cq_t = whpool.tile([ROPE, S], F32, tag="cq")
                    sq_t = whpool.tile([ROPE, S], F32, tag="sq")
                    nc.gpsimd.dma_start(out=cq_t[:], in_=cq[:, :])
                    nc.gpsimd.dma_start(out=sq_t[:], in_=sq[:, :])
                    for r in range(NCORES):
                        csl = slice(r * SLC, (r + 1) * SLC)
                        qac = []
                        for j in range(NQB):
                            t = apool.tile([P, SLC], FR, tag=f"qa{j}", name=f"qac{j}_{r}")
                            nc.gpsimd.dma_start(out=t[:], in_=cc_q_out[r, j * P:(j + 1) * P, :])
                            qac.append(t)
                        for h in range(NHC):
                            qoff = h * QHD
                            ps = ps2pool.tile([P, SLC], F32, tag="p2", name=f"p2q{h}_{r}")
                            for j in range(NQB):
                                nc.tensor.matmul(ps[:], lhsT=fr(wqb_t[:, j, qoff:qoff + NOPE]),
                                                 rhs=qac[j][:],
                                                 start=(j == 0), stop=(j == NQB - 1))
                            qns = ropepool.tile([NOPE, SLC], F32, tag="qns", name=f"qns{h}_{r}")
                            nc.scalar.copy(qns[:], ps[:])
                            nc.gpsimd.dma_start(out=qnT[h * NOPE:(h + 1) * NOPE, csl], in_=qns[:])
                            ps64 = ps2pool.tile([ROPE, SLC], F32, tag="p64", name=f"p64q{h}_{r}")
                            for j in range(NQB):
                                nc.tensor.matmul(ps64[:], lhsT=fr(wqb_t[:, j, qoff + NOPE:qoff + QHD]),
                                                 rhs=qac[j][:],
                                                 start=(j == 0), stop=(j == NQB - 1))
                            qraw = ropepool.tile([ROPE, SLC], F32, tag="qraw", name=f"qraw{h}_{r}")
                            nc.scalar.copy(qraw[:], ps64[:])
                            qsw = ropepool.tile([ROPE, SLC], F32, tag="qsw", name=f"qsw{h}_{r}")
                            nc.gpsimd.dma_start(out=qsw[0:32, :], in_=qraw[32:64, :])
                            nc.gpsimd.dma_start(out=qsw[32:64, :], in_=qraw[0:32, :])
                            qa_ = ropepool.tile([ROPE, SLC], F32, tag="qa_", name=f"qa_{h}_{r}")
                            nc.vector.tensor_mul(qa_[:], qraw[:], cq_t[:, csl])
                            qb_ = ropepool.tile([ROPE, SLC], F32, tag="qb_", name=f"qb_{h}_{r}")
                            nc.vector.tensor_mul(qb_[:], qsw[:], sq_t[:, csl])
                            qrs = ropepool.tile([ROPE, SLC], F32, tag="qrs", name=f"qrs{h}_{r}")
                            nc.vector.tensor_add(qrs[:], qa_[:], qb_[:])
                            nc.gpsimd.dma_start(out=qrT[h * ROPE:(h + 1) * ROPE, csl], in_=qrs[:])

                # ------------- Phase 2b: attention
                with (
                    tc.tile_pool(name="att", bufs=2) as attpool,
                    tc.tile_pool(name="den", bufs=1) as denpool,
                    tc.tile_pool(name="ps_o", bufs=1, space="PSUM") as psopool,
                    tc.tile_pool(name="ps_l", bufs=2, space="PSUM") as pslpool,
                    tc.tile_pool(name="ps_d", bufs=1, space="PSUM") as psdpool,
                ):
                    for qc in range(NSC):
                        qsl = slice(qc * SC, (qc + 1) * SC)
                        kb_hi = (qc * 4 + 4) if causal else NKB
                        ops = [psopool.tile([VD, SC], F32, tag=f"o{h}", name=f"o{h}_{qc}") for h in range(NHC)]
                        dens = [denpool.tile([P, SC], FR, tag=f"d{h}", name=f"d{h}_{qc}") for h in range(NHC)]
                        qn_s, qr_s = [], []
                        for h in range(NHC):
                            qt = denpool.tile([NOPE, SC], FR, tag=f"qns{h}", name=f"qnl{h}_{qc}")
                            nc.gpsimd.dma_start(out=qt[:], in_=qnT[h * NOPE:(h + 1) * NOPE, qsl])
                            qn_s.append(qt)
                            qt2 = denpool.tile([ROPE, SC], FR, tag=f"qrs{h}", name=f"qrl{h}_{qc}")
                            nc.gpsimd.dma_start(out=qt2[:], in_=qrT[h * ROPE:(h + 1) * ROPE, qsl])
                            qr_s.append(qt2)
                        for kb in range(kb_hi):
                            ksl = slice(kb * P, (kb + 1) * P)
                            mt = attpool.tile([P, SC], F32, tag="mt", name=f"mt{qc}_{kb}")
                            nc.gpsimd.dma_start(out=mt[:], in_=maskT[ksl, qsl])
                            for h in range(NHC):
                                pl = pslpool.tile([P, SC], F32, tag="pl", name=f"pl{qc}_{kb}_{h}")
                                nc.tensor.matmul(pl[:], lhsT=KN[h][:, ksl], rhs=qn_s[h][:],
                                                 start=True, stop=False)
                                nc.tensor.matmul(pl[:], lhsT=kpe_sb[:, ksl], rhs=qr_s[h][:],
                                                 start=False, stop=True)
                                pe_ = attpool.tile([P, SC], F32, tag="pe", name=f"pe{qc}_{kb}_{h}")
                                nc.vector.tensor_add(pe_[:], pl[:], mt[:])
                                px = attpool.tile([P, SC], FR, tag="px", name=f"px{qc}_{kb}_{h}")
                                nc.scalar.activation(px[:], pe_[:], AF.Exp)
                                if kb == 0:
                                    nc.vector.tensor_copy(dens[h][:], px[:])
                                else:
                                    nc.vector.tensor_add(dens[h][:], dens[h][:], px[:])
                                nc.tensor.matmul(ops[h][:], lhsT=fr(V[kb][:, h, :]), rhs=px[:],
                                                 start=(kb == 0), stop=(kb == kb_hi - 1))
                        for h in range(NHC):
                            dps = psdpool.tile([1, SC], F32, tag="dps", name=f"dps{qc}_{h}")
                            nc.tensor.matmul(dps[:], lhsT=ones_t[:], rhs=dens[h][:],
                                             start=True, stop=True)
                            dsb = attpool.tile([1, SC], F32, tag="dsb", name=f"dsb{qc}_{h}")
                            nc.scalar.copy(dsb[:], dps[:])
                            rcp = attpool.tile([1, SC], FR, tag="rcp", name=f"rcp{qc}_{h}")
                            with nc.allow_low_precision(reason="f32r rounding for broadcast matmul"):
                                nc.vector.reciprocal(rcp[:], dsb[:])
                            bps2 = psdpool.tile([VD, SC], F32, tag="bps2", name=f"bps2{qc}_{h}")
                            nc.tensor.matmul(bps2[:], lhsT=ones_row[:], rhs=rcp[:],
                                             start=True, stop=True)
                            rbb = attpool.tile([VD, SC], F32, tag="rbb", name=f"rbb{qc}_{h}")
                            nc.scalar.copy(rbb[:], bps2[:])
                            on_ = attpool.tile([VD, SC], F32, tag="on", name=f"on{qc}_{h}")
                            nc.vector.tensor_mul(on_[:], ops[h][:], rbb[:])
                            nc.gpsimd.dma_start(out=onT[h * VD:(h + 1) * VD, qsl], in_=on_[:])
                kv2pool.__exit__(None, None, None)

            # ------------- Phase 3: output projection (partial over head slice)
            with (
                tc.tile_pool(name="wo", bufs=1) as wopool,
                tc.tile_pool(name="oc", bufs=1) as ocpool,
                tc.tile_pool(name="oo", bufs=3) as oopool,
                tc.tile_pool(name="po", bufs=3, space="PSUM") as popool,
            ):
                wo_t = wopool.tile([P, NKVB, H], FR, tag="wo")
                nc.gpsimd.dma_start(out=wo_t[:], in_=Wo3[:, :, :])
                for sc in range(NSC):
                    ssl = slice(sc * SC, (sc + 1) * SC)
                    ocs = []
                    for j in range(NKVB):
                        t = ocpool.tile([P, SC], FR, tag=f"oc{j}", name=f"oc{j}_{sc}")
                        nc.gpsimd.dma_start(out=t[:], in_=onT[j * P:(j + 1) * P, ssl])
                        ocs.append(t)
                    for ho in range(H // P):
                        ps = popool.tile([P, SC], F32, tag="po", name=f"po{sc}_{ho}")
                        for j in range(NKVB):
                            nc.tensor.matmul(ps[:], lhsT=fr(wo_t[:, j, ho * P:(ho + 1) * P]),
                                             rhs=ocs[j][:], start=(j == 0), stop=(j == NKVB - 1))
                        ot = oopool.tile([P, SC], F32, tag="ot", name=f"ot{sc}_{ho}")
                        nc.scalar.copy(ot[:], ps[:])
                        nc.gpsimd.dma_start(out=outT[ho * P:(ho + 1) * P, ssl], in_=ot[:])

    split_multiwaits(nc)
    return nc


def _pack_front(WqaT, WkvaT):
    """[4096, 1536+576] -> [128, 17*32, 128], zero-padded rope block."""
    Wfull = np.concatenate([WqaT, WkvaT], axis=1)
    out = np.zeros((P, N_FB * N_KI, P), np.float32)
    off = 0
    for fb, w in enumerate(FB_W):
        blk = Wfull[:, off:off + w].reshape(N_KI, P, w).transpose(1, 0, 2)
        out[:, fb * N_KI:(fb + 1) * N_KI, :w] = blk
        off += w
    return np.ascontiguousarray(out.reshape(P, -1))


def _pack_k(WT, nhw):
    """[K, nhw] -> [128, (K//128)*nhw]: k-tile-major packing of a T-layout weight."""
    K = WT.shape[0]
    t = WT.reshape(K // P, P, nhw).transpose(1, 0, 2).reshape(P, (K // P) * nhw)
    return np.ascontiguousarray(t, np.float32)


def _rope_tables():
    inv = 1.0 / (BASE ** (np.arange(0, ROPE, 2, dtype=np.float64) / ROPE))
    t = np.arange(S, dtype=np.float64)
    fr_ = np.outer(t, inv)
    emb = np.concatenate([fr_, fr_], axis=1)
    cos = np.cos(emb).T.astype(np.float32)
    sin = np.sin(emb).T.astype(np.float32)
    ssin = sin.copy()
    ssin[:32] *= -1.0
    return cos, ssin


def kernel(hidden_states, attention_mask, Wqa, qa_ln_w, Wqb, Wkva, kva_ln_w, Wkvb, Wo):
    hidden_states = np.asarray(hidden_states, np.float32)
    attention_mask = np.asarray(attention_mask, np.float32)
    Wqa = np.asarray(Wqa, np.float32)
    Wqb = np.asarray(Wqb, np.float32)
    Wkva = np.asarray(Wkva, np.float32)
    Wkvb = np.asarray(Wkvb, np.float32)
    Wo = np.asarray(Wo, np.float32)
    qa_ln_w = np.asarray(qa_ln_w, np.float32)
    kva_ln_w = np.asarray(kva_ln_w, np.float32)

    mask = attention_mask[0, 0]
    tril = np.tril(np.ones((S, S), bool))
    causal = bool(np.array_equal(mask, np.where(tril, 0.0, -1e9).astype(np.float32)))

    hT = np.ascontiguousarray(hidden_states[0].T)
    maskT = np.ascontiguousarray(mask.T)
    Wp = _pack_front(np.ascontiguousarray(Wqa.T), np.ascontiguousarray(Wkva.T))
    cos, ssin = _rope_tables()

    Wqb_eff = (Wqb * qa_ln_w[None, :]).astype(np.float32) * np.float32(SCALE)
    Wkvb_eff = (Wkvb * kva_ln_w[None, :]).astype(np.float32)

    in_maps = []
    for c in range(NCORES):
        hsl = slice(c * NHC * QHD, (c + 1) * NHC * QHD)
        ksl = slice(c * NHC * (NOPE + VD), (c + 1) * NHC * (NOPE + VD))
        osl = slice(c * NHC * VD, (c + 1) * NHC * VD)
        in_maps.append({
            "hT": hT, "maskT": maskT, "Wp": Wp,
            "hTs": np.ascontiguousarray(hT[:, c * SLC:(c + 1) * SLC]),
            "Wqb_p": _pack_k(np.ascontiguousarray(Wqb_eff[hsl].T), NHC * QHD),
            "Wkvb_p": _pack_k(np.ascontiguousarray(Wkvb_eff[ksl].T), NHC * (NOPE + VD)),
            "Wo_p": _pack_k(np.ascontiguousarray(Wo[:, osl].T), H),
            "cq": cos, "sq": ssin,
        })

    nc = build(causal)
    trace = bool(os.environ.get("KPROF"))
    kw = {}
    td = os.environ.get("KPROF_DIR")
    if trace and td:
        os.makedirs(td, exist_ok=True)
        kw["tmpdir"] = td
    res = run_bass_kernel_spmd(nc, in_maps, list(range(NCORES)), trace=trace, **kw)
    if trace:
        print(f"HW exec time: {res.exec_time_ns} ns (mean {res.mean_exec_time_ns}, "
              f"max core {res.max_exec_time_core_id})")
    acc = res.results[0]["outT"].copy()
    for c in range(1, NCORES):
        acc += res.results[c]["outT"]
    return np.ascontiguousarray(acc.T)[None, :, :].astype(np.float32)



# revision 30
# speedup vs baseline: 2.1037x; 2.1037x over previous
"""DeepseekV2 MLA attention (B=1, S=2048, H=4096, NH=32) on 8 TRN2 cores. v2.

Sharding: tensor-parallel over heads (4 heads/core).  The whole low-rank
front (q_a AND kv_a, raw, un-normalized) runs data-parallel over sequence:
each core computes its 256-token slice, applies rope to k_pe locally, and
AllGathers bf16 payloads plus f32 square-sum rows (RMSNorm is deferred: the
per-token 1/rms scale commutes through the up-projections and is applied
post-matmul, before softmax).  Each core then builds K_nope/V for its 4
heads (SBUF-resident), and runs a per-512-query-chunk pipeline of
Q-projection+rope -> causal flash-style attention -> partial output
projection (its head slice of Wo); the host sums the 8 partials.

All heavy operands move as bf16 (weights pre-packed/pre-cast on the host,
activation payloads cast on device); PE accumulates in f32.  Attention
probabilities stay f32r.  Causal masking loads only the 4 distinct diagonal
128x512 tiles; fully-below-diagonal blocks skip the mask add and
above-diagonal blocks are skipped entirely.
"""

import os
import numpy as np
import ml_dtypes

import concourse.bass as bass
import concourse.mybir as mybir
from concourse.tile import TileContext
import concourse.bass_utils as bass_utils
from concourse.bass_utils import run_bass_kernel_spmd

bass_utils.upload_artifacts = lambda tmpdir: tmpdir  # no artifact bucket here

S = 2048
H = 4096
NCORES = 8
NHC = 4            # heads per core
NOPE, ROPE, VD = 128, 64, 128
QHD = NOPE + ROPE  # 192
QLR, KVLR = 1536, 512
BASE = 10000.0
EPS = 1e-6
SCALE = QHD ** -0.5
P = 128
SC = 512           # query chunk
SLC = S // NCORES  # 256, per-core front slice
NQC = S // SC      # 4 query chunks
NKB = S // P       # 16 key blocks
FR = mybir.dt.float32r
F32 = mybir.dt.float32
BF = mybir.dt.bfloat16
AF = mybir.ActivationFunctionType

N_KI = H // P      # 32 front contraction tiles
NQB = QLR // P     # 12
NKVB = KVLR // P   # 4
# front output blocks, kv first: 4x128 kvn, 64 rope, 12x128 q_a
FB_W = [P] * NKVB + [ROPE] + [P] * NQB
N_FB = len(FB_W)   # 17
RB = NKVB          # index of the rope block


def split_multiwaits(nc, cap=1):
    """Allow only `cap` sync-waits per instruction; spill extras onto
    same-engine NoOps inserted just before the instruction."""
    for f in nc.m.functions:
        for b in f.blocks:
            li = b.instructions
            out = []
            changed = False
            for inst in list(li):
                si = getattr(inst, "sync_info", None)
                waits = list(si.on_wait) if si is not None and si.on_wait else []
                if len(waits) > cap:
                    changed = True
                    extra, keep = waits[:-cap], waits[-cap:]
                    for j in range(0, len(extra), cap):
                        out.append(mybir.InstNoOp(
                            name=nc.get_next_instruction_name(),
                            engine=inst.engine, ins=[], outs=[],
                            sync_info=mybir.SyncInfo(
                                on_wait=extra[j:j + cap], on_update=[]),
                            bass_nofuse=True,
                        ))
                    inst.sync_info = mybir.SyncInfo(
                        on_wait=keep, on_update=list(si.on_update))
                out.append(inst)
            if changed:
                li[:] = out


def build(causal: bool) -> bass.Bass:
    nc = bass.Bass()
    hTs_p = nc.declare_dram_parameter("hTs_p", [P, N_KI * SLC], BF, isOutput=False)
    Wp = nc.declare_dram_parameter("Wp", [P, N_FB * N_KI * P], BF, isOutput=False)
    Wqb_p = nc.declare_dram_parameter("Wqb_p", [P, NQB * NHC * QHD], BF, isOutput=False)
    Wkvb_p = nc.declare_dram_parameter("Wkvb_p", [P, NKVB * NHC * (NOPE + VD)], BF, isOutput=False)
    Wo_p = nc.declare_dram_parameter("Wo_p", [P, NKVB * H], BF, isOutput=False)
    cq2 = nc.declare_dram_parameter("cq2", [P, S], F32, isOutput=False)
    sq2 = nc.declare_dram_parameter("sq2", [P, S], F32, isOutput=False)
    cqk = nc.declare_dram_parameter("cqk", [ROPE, SLC], F32, isOutput=False)
    sqk = nc.declare_dram_parameter("sqk", [ROPE, SLC], F32, isOutput=False)
    maskDg = nc.declare_dram_parameter("maskDg", [P, P], BF, isOutput=False)
    maskT = nc.declare_dram_parameter("maskT", [S, S], F32, isOutput=False)
    outT = nc.declare_dram_parameter("outT", [H, S], F32, isOutput=True)

    Wp3 = Wp.rearrange("p (fk w) -> p fk w", w=P)          # [P, 17*32, 128]

    def fr(ap):
        return ap.bitcast(FR)

    KVROWS = KVLR + ROPE  # 576

    with TileContext(nc) as tc:
        with (
            tc.tile_pool(name="dram", bufs=1, space="DRAM") as dpool,
            tc.tile_pool(name="const", bufs=1) as cpool,
            tc.tile_pool(name="wbig", bufs=1) as wbpool,
            tc.tile_pool(name="kv", bufs=1) as kvpool,
        ):
            # payload rows: [data | f32 square-sums bitcast as 2 bf16 rows]
            cc_q_in = dpool.tile([QLR + 2, SLC], BF)
            cc_q_out = dpool.tile([NCORES, QLR + 2, SLC], BF, addr_space="Shared")
            cc_kv_in = dpool.tile([KVROWS + 2, SLC], BF)
            cc_kv_out = dpool.tile([NCORES, KVROWS + 2, SLC], BF, addr_space="Shared")

            ones_f = cpool.tile([P, 1], F32)
            nc.vector.memset(ones_f[:], 1.0)
            ones_rf = cpool.tile([1, P], F32)
            nc.vector.memset(ones_rf[:], 1.0)
            ones_t = cpool.tile([P, 1], FR)
            nc.scalar.copy(ones_t[:], ones_f[:])
            ones_row = cpool.tile([1, P], FR)
            nc.scalar.copy(ones_row[:], ones_rf[:])
            # PE warmup: trip the HAM busy window before real work arrives so
            # the front starts at 2.4 GHz (junk matmuls into a scratch bank)
            warm_rhs = cpool.tile([1, SC], FR)
            nc.vector.memset(warm_rhs.bitcast(F32)[:], 1.0)

            # persistent attention operands
            KN = [kvpool.tile([P, S], BF, tag=f"kn{h}") for h in range(NHC)]
            V = [kvpool.tile([P, NHC * VD], FR, tag=f"v{kb}") for kb in range(NKB)]
            kpe_t = kvpool.tile([ROPE, NCORES, SLC], BF, tag="kpe")
            s_rowF = kvpool.tile([1, S], FR, tag="srow")

            # ---------------- Phase F: raw front (kv blocks first) + rope(k)
            with (
                tc.tile_pool(name="hts", bufs=1) as hpool,
                tc.tile_pool(name="wfr", bufs=4) as wpool,
                tc.tile_pool(name="raw", bufs=3) as rpool,
                tc.tile_pool(name="sqr", bufs=2) as sqpool,
                tc.tile_pool(name="ccb", bufs=5) as ccpool,
                tc.tile_pool(name="krope", bufs=1) as krpool,
                tc.tile_pool(name="psf", bufs=2, space="PSUM") as psf,
                tc.tile_pool(name="pswarm", bufs=1, space="PSUM") as pswarm,
                tc.tile_pool(name="pssum", bufs=1, space="PSUM") as pssum,
            ):
                hts = hpool.tile([P, N_KI, SLC], BF)
                nc.sync.dma_start(out=hts[:, :, :], in_=hTs_p[:, :])
                warm_ps = pswarm.tile([P, SC], F32, tag="warm")
                for i in range(24):
                    nc.tensor.matmul(warm_ps[:], lhsT=ones_row[:], rhs=warm_rhs[:],
                                     start=(i == 0), stop=(i == 23))
                sq_kv = pssum.tile([1, SLC], F32, tag="sqkv")
                sq_q = pssum.tile([1, SLC], F32, tag="sqq")
                for fb in range(N_FB):
                    w = FB_W[fb]
                    wt = wpool.tile([P, N_KI, P], BF, tag="w", name=f"w{fb}")
                    weng = nc.sync if fb % 2 == 0 else nc.scalar
                    weng.dma_start(out=wt[:], in_=Wp3[:, fb * N_KI:(fb + 1) * N_KI, :])
                    ps = psf.tile([P, SLC], F32, tag="ps", name=f"psf{fb}")
                    for ki in range(N_KI):
                        nc.tensor.matmul(ps[:w, :], lhsT=wt[:, ki, :w], rhs=hts[:, ki, :],
                                         start=(ki == 0), stop=(ki == N_KI - 1))
                    raw = rpool.tile([P, SLC], F32, tag="r", name=f"raw{fb}")
                    nc.scalar.copy(raw[:w, :], ps[:w, :])
                    if fb != RB:
                        sqt = sqpool.tile([P, SLC], FR, tag="sqt", name=f"sqt{fb}")
                        nc.vector.tensor_mul(sqt[:], raw[:], raw[:])
                        if fb < RB:
                            nc.tensor.matmul(sq_kv[:], lhsT=ones_t[:], rhs=sqt[:],
                                             start=(fb == 0), stop=(fb == NKVB - 1))
                        else:
                            nc.tensor.matmul(sq_q[:], lhsT=ones_t[:], rhs=sqt[:],
                                             start=(fb == RB + 1), stop=(fb == N_FB - 1))
                        ccb = ccpool.tile([P, SLC], BF, tag="cc", name=f"cc{fb}")
                        nc.vector.tensor_copy(ccb[:], raw[:])
                        if fb < RB:
                            nc.gpsimd.dma_start(out=cc_kv_in[fb * P:(fb + 1) * P, :], in_=ccb[:])
                        else:
                            q0 = (fb - RB - 1) * P
                            nc.gpsimd.dma_start(out=cc_q_in[q0:q0 + P, :], in_=ccb[:])
                    else:
                        # rope on raw k_pe rows [0:64] of this block
                        ck_t = krpool.tile([ROPE, SLC], F32, tag="ck")
                        sk_t = krpool.tile([ROPE, SLC], F32, tag="sk")
                        nc.sync.dma_start(out=ck_t[:], in_=cqk[:, :])
                        nc.sync.dma_start(out=sk_t[:], in_=sqk[:, :])
                        ksw = krpool.tile([ROPE, SLC], F32, tag="ksw")
                        nc.sync.dma_start(out=ksw[0:32, :], in_=raw[32:64, :])
                        nc.sync.dma_start(out=ksw[32:64, :], in_=raw[0:32, :])
                        ka = krpool.tile([ROPE, SLC], F32, tag="ka")
                        nc.vector.tensor_mul(ka[:], raw[:ROPE, :], ck_t[:])
                        kb_ = krpool.tile([ROPE, SLC], F32, tag="kb")
                        nc.vector.tensor_mul(kb_[:], ksw[:], sk_t[:])
                        ko = krpool.tile([ROPE, SLC], BF, tag="ko")
                        nc.vector.tensor_add(ko[:], ka[:], kb_[:])
                        nc.gpsimd.dma_start(out=cc_kv_in[KVLR:KVROWS, :], in_=ko[:])
                        # kv square-sums: pack f32 row as 2 bf16 rows, one AG
                        kvs = krpool.tile([1, SLC], F32, tag="kvs")
                        nc.scalar.copy(kvs[:], sq_kv[:])
                        nc.gpsimd.dma_start(out=cc_kv_in[KVROWS:KVROWS + 2, :],
                                            in_=kvs.bitcast(BF)[:])
                        nc.gpsimd.collective_compute(
                            "AllGather", mybir.AluOpType.bypass,
                            replica_groups=[list(range(NCORES))],
                            ins=[cc_kv_in.opt()], outs=[cc_kv_out.opt()])
                qs = krpool.tile([1, SLC], F32, tag="qs")
                nc.scalar.copy(qs[:], sq_q[:])
                nc.gpsimd.dma_start(out=cc_q_in[QLR:QLR + 2, :],
                                    in_=qs.bitcast(BF)[:])
                nc.gpsimd.collective_compute(
                    "AllGather", mybir.AluOpType.bypass,
                    replica_groups=[list(range(NCORES))],
                    ins=[cc_q_in.opt()], outs=[cc_q_out.opt()])

            # prefetch big weights (scalar HWDGE ring; lands right after front)
            Wkvb_t = wbpool.tile([P, NKVB, NHC * (NOPE + VD)], BF, tag="wkvb")
            nc.scalar.dma_start(out=Wkvb_t[:], in_=Wkvb_p[:, :])
            Wqb_t = wbpool.tile([P, NQB, NHC * QHD], BF, tag="wqb")
            nc.scalar.dma_start(out=Wqb_t[:], in_=Wqb_p[:, :])
            Wo_t = wbpool.tile([P, NKVB, H], BF, tag="wo")
            nc.scalar.dma_start(out=Wo_t[:], in_=Wo_p[:, :])
            maskDg_t = wbpool.tile([P, P], BF, tag="maskdg")
            nc.scalar.dma_start(out=maskDg_t[:], in_=maskDg[:, :])

            # ---------------- Phase KV: K_nope / V for this core's heads
            with (
                tc.tile_pool(name="kvg", bufs=1) as gpool,
                tc.tile_pool(name="tsc", bufs=1) as tpool,
                tc.tile_pool(name="ps2", bufs=2, space="PSUM") as ps2,
            ):
                kvg = [gpool.tile([P, NCORES, SLC], BF, tag=f"g{j}") for j in range(NKVB)]
                for j in range(NKVB):
                    for r in range(NCORES):
                        nc.gpsimd.dma_start(
                            out=kvg[j][:, r, :],
                            in_=cc_kv_out[r, j * P:(j + 1) * P, :])
                for r in range(NCORES):
                    nc.gpsimd.dma_start(out=kpe_t[:, r, :],
                                        in_=cc_kv_out[r, KVLR:KVROWS, :])
                # per-key scale, column layout [1, S] for KN bcast
                # rank-major [8,256]: recip runs on 8 DVE lanes (~1.6us, not
                # 12.8us single-lane); flatten to the [1,S] row with one DMA
                t_row = tpool.tile([NCORES, SLC], F32, tag="trow", name="t_row")
                tmp_r = tpool.tile([NCORES, SLC], F32, tag="tmp", name="tmp_r")
                nc.scalar.dma_start(out=t_row[:], in_=cc_kv_out[:, KVROWS:KVROWS + 2, :].bitcast(F32))
                nc.scalar.activation(tmp_r[:], t_row[:], AF.Copy, scale=1.0 / KVLR, bias=EPS)
                nc.vector.reciprocal(t_row[:], tmp_r[:])
                t8F = tpool.tile([NCORES, SLC], FR, tag="t8f", name="t8F")
                nc.scalar.activation(t8F[:], t_row[:], AF.Sqrt)
                t_rowF = tpool.tile([1, S], FR, tag="trf", name="t_rowF")
                nc.scalar.dma_start(out=t_rowF[:], in_=t8F[:])
                # per-key scale, per-partition layout [P, NKB] for V scaling
                t_colT = tpool.tile([P, NKB], F32, tag="tcol")
                for kb in range(NKB):
                    r, row = kb // 2, kb % 2
                    nc.scalar.dma_start(
                        out=t_colT[:, kb:kb + 1],
                        in_=cc_kv_out[r, KVROWS + row, :].bitcast(F32))
                msc = tpool.tile([P, NKB], F32, tag="msc")
                nc.scalar.activation(msc[:], t_colT[:], AF.Copy, scale=1.0 / KVLR, bias=EPS)
                rcc = tpool.tile([P, NKB], F32, tag="rcc")
                nc.vector.reciprocal(rcc[:], msc[:])
                t_colF = tpool.tile([P, NKB], F32, tag="tcf")
                nc.scalar.activation(t_colF[:], rcc[:], AF.Sqrt)
                # q-token scale row (for later phases); reuses t scratch after
                # tbt is built below (Tile inserts the WAR deps)
                s_scr = tpool.tile([NCORES, SLC], F32, tag="sscr", name="s_scr")
                nc.scalar.dma_start(out=s_scr[:], in_=cc_q_out[:, QLR:QLR + 2, :].bitcast(F32))
                nc.scalar.activation(s_scr[:], s_scr[:], AF.Copy, scale=1.0 / QLR, bias=EPS)
                nc.vector.reciprocal(s_scr[:], s_scr[:])
                s8F = tpool.tile([NCORES, SLC], FR, tag="s8f", name="s8F")
                nc.scalar.activation(s8F[:], s_scr[:], AF.Sqrt)
                nc.scalar.dma_start(out=s_rowF[:], in_=s8F[:])

                # t broadcast [P, S] (f32) for KN column scaling
                tbt = tpool.tile([P, S], F32, tag="tbt")
                for sc4 in range(NQC):
                    ssl = slice(sc4 * SC, (sc4 + 1) * SC)
                    tb_ps = ps2.tile([P, SC], F32, tag="tb", name=f"tb{sc4}")
                    nc.tensor.matmul(tb_ps[:], lhsT=ones_row[:], rhs=t_rowF[:, ssl],
                                     start=True, stop=True)
                    nc.scalar.copy(tbt[:, ssl], tb_ps[:])
                # K_nope [d, S] per head (bf16, scaled)
                for h in range(NHC):
                    for sc4 in range(NQC):
                        ssl = slice(sc4 * SC, (sc4 + 1) * SC)
                        ps = ps2.tile([P, SC], F32, tag="pk", name=f"pk{h}_{sc4}")
                        for j in range(NKVB):
                            nc.tensor.matmul(
                                ps[:], lhsT=Wkvb_t[:, j, h * P:(h + 1) * P],
                                rhs=kvg[j][:, 2 * sc4:2 * sc4 + 2, :],
                                start=(j == 0), stop=(j == NKVB - 1))
                        nc.vector.tensor_mul(KN[h][:, ssl], ps[:], tbt[:, ssl])
                # V [keys, 4 heads * VD] per key block (f32r, scaled)
                for kb in range(NKB):
                    r, half = kb // 2, (kb % 2) * P
                    ps = ps2.tile([P, NHC * VD], F32, tag="pv", name=f"pv{kb}")
                    for j in range(NKVB):
                        nc.tensor.matmul(
                            ps[:], lhsT=kvg[j][:, r, half:half + P],
                            rhs=Wkvb_t[:, j, NHC * NOPE:NHC * (NOPE + VD)],
                            start=(j == 0), stop=(j == NKVB - 1))
                    nc.vector.tensor_scalar_mul(V[kb][:], ps[:], t_colF[:, kb:kb + 1])

            # ---------------- Phase A: per-query-chunk pipeline
            with (
                tc.tile_pool(name="qag", bufs=2) as qapool,
                tc.tile_pool(name="qn", bufs=1) as qnpool,
                tc.tile_pool(name="ropet", bufs=2) as ropepool,
                tc.tile_pool(name="att", bufs=2) as attpool,
                tc.tile_pool(name="rcpp", bufs=1) as rcppool,
                tc.tile_pool(name="px", bufs=4) as pxpool,
                tc.tile_pool(name="dens", bufs=1) as denspool,
                tc.tile_pool(name="o", bufs=1) as opool,
                tc.tile_pool(name="oo", bufs=3) as oopool,
                tc.tile_pool(name="mt", bufs=2) as mtpool,
                tc.tile_pool(name="psq", bufs=2, space="PSUM") as psq,
                tc.tile_pool(name="psl", bufs=2, space="PSUM") as psl,
                tc.tile_pool(name="psd", bufs=1, space="PSUM") as psd,
                tc.tile_pool(name="psav", bufs=1, space="PSUM") as psav,
            ):
                cc_q_r = cc_q_out[:, :QLR, :].rearrange("r (j p) c -> r p j c", p=P)

                def qload(qc):
                    qsl = slice(qc * SC, (qc + 1) * SC)
                    qag_all = qapool.tile([P, NQB, 2, SLC], BF, tag="qaall",
                                          name=f"qaall_{qc}")
                    for ri in range(2):
                        nc.gpsimd.dma_start(
                            out=qag_all[:, :, ri, :],
                            in_=cc_q_r[2 * qc + ri, :, :, :])
                    qag = [qag_all[:, j, :, :] for j in range(NQB)]
                    cq_t = ropepool.tile([P, SC], F32, tag="cqs", name=f"cq{qc}")
                    sq_t = ropepool.tile([P, SC], F32, tag="sqs", name=f"sq{qc}")
                    nc.sync.dma_start(out=cq_t[:], in_=cq2[:, qsl])
                    nc.sync.dma_start(out=sq_t[:], in_=sq2[:, qsl])
                    return qag, cq_t, sq_t

                def qproj(qc, loaded):
                    qsl = slice(qc * SC, (qc + 1) * SC)
                    qag, cq_t, sq_t = loaded
                    sb_ps = psq.tile([P, SC], F32, tag="pq", name=f"sb{qc}")
                    nc.tensor.matmul(sb_ps[:], lhsT=ones_row[:], rhs=s_rowF[:, qsl],
                                     start=True, stop=True)
                    s_bt = ropepool.tile([P, SC], F32, tag="sbt", name=f"sbt{qc}")
                    nc.scalar.copy(s_bt[:], sb_ps[:])
                    qn_t, qr_t = [], [None] * NHC
                    for pr in range(2):
                        c0 = NHC * P + pr * P
                        ps = psq.tile([P, SC], F32, tag="pq", name=f"pqr{pr}_{qc}")
                        for j in range(NQB):
                            nc.tensor.matmul(ps[:], lhsT=Wqb_t[:, j, c0:c0 + P],
                                             rhs=qag[j],
                                             start=(j == 0), stop=(j == NQB - 1))
                        qraw = ropepool.tile([P, SC], F32, tag="qraw", name=f"qraw{pr}_{qc}")
                        nc.scalar.copy(qraw[:], ps[:])
                        qsw = ropepool.tile([P, SC], F32, tag="qsw", name=f"qsw{pr}_{qc}")
                        for hh in range(2):
                            b0 = hh * ROPE
                            nc.sync.dma_start(out=qsw[b0:b0 + 32, :], in_=qraw[b0 + 32:b0 + 64, :])
                            nc.sync.dma_start(out=qsw[b0 + 32:b0 + 64, :], in_=qraw[b0:b0 + 32, :])
                        qa_ = ropepool.tile([P, SC], F32, tag="qa_", name=f"qa_{pr}_{qc}")
                        nc.vector.tensor_mul(qa_[:], qraw[:], cq_t[:])
                        qb_ = ropepool.tile([P, SC], F32, tag="qb_", name=f"qb_{pr}_{qc}")
                        nc.vector.tensor_mul(qb_[:], qsw[:], sq_t[:])
                        qs_ = ropepool.tile([P, SC], F32, tag="qs_", name=f"qs_{pr}_{qc}")
                        nc.vector.tensor_add(qs_[:], qa_[:], qb_[:])
                        qrb = ropepool.tile([P, SC], BF, tag="qrb", name=f"qrb{pr}_{qc}")
                        nc.vector.tensor_mul(qrb[:], qs_[:], s_bt[:])
                        for hh in range(2):
                            h = 2 * pr + hh
                            qr = qnpool.tile([ROPE, SC], BF, tag=f"qr{h}", name=f"qr{h}_{qc}")
                            nc.sync.dma_start(out=qr[:], in_=qrb[hh * ROPE:(hh + 1) * ROPE, :])
                            qr_t[h] = qr
                    for h in range(NHC):
                        ps = psq.tile([P, SC], F32, tag="pq", name=f"pqn{h}_{qc}")
                        for j in range(NQB):
                            nc.tensor.matmul(ps[:], lhsT=Wqb_t[:, j, h * P:(h + 1) * P],
                                             rhs=qag[j],
                                             start=(j == 0), stop=(j == NQB - 1))
                        qn = qnpool.tile([P, SC], BF, tag=f"qn{h}", name=f"qn{h}_{qc}")
                        nc.vector.tensor_mul(qn[:], ps[:], s_bt[:])
                        qn_t.append(qn)
                    return qn_t, qr_t

                qldcur = qload(0)
                qcur = qproj(0, qldcur)
                for qc in range(NQC):
                    qsl = slice(qc * SC, (qc + 1) * SC)
                    qn_t, qr_t = qcur
                    kb_hi = 4 * (qc + 1) if causal else NKB
                    if qc + 1 < NQC:
                        qldnext = qload(qc + 1)
                    avs = [psav.tile([VD, SC], F32, tag=f"av{h}", name=f"av{h}_{qc}")
                           for h in range(NHC)]
                    dens = [denspool.tile([P, SC], FR, tag=f"d{h}", name=f"d{h}_{qc}")
                            for h in range(NHC)]
                    for kb in range(kb_hi):
                        r, half = kb // 2, (kb % 2) * P
                        if causal:
                            m = kb - 4 * qc
                            diag = m >= 0
                            q0 = m * P if diag else 0
                        else:
                            diag, q0 = True, 0
                            mtt = mtpool.tile([P, SC], F32, tag="mt", name=f"mt{qc}_{kb}")
                            nc.sync.dma_start(out=mtt[:], in_=maskT[kb * P:(kb + 1) * P, qsl])
                        pxs = []
                        for h in range(NHC):
                            pl = psl.tile([P, SC], F32, tag="pl", name=f"pl{qc}_{kb}_{h}")
                            nc.tensor.matmul(pl[:, q0:], lhsT=KN[h][:, kb * P:(kb + 1) * P],
                                             rhs=qn_t[h][:, q0:], start=True, stop=False)
                            nc.tensor.matmul(pl[:, q0:], lhsT=kpe_t[:, r, half:half + P],
                                             rhs=qr_t[h][:, q0:], start=False, stop=True)
                            px = pxpool.tile([P, SC], BF, tag="px", name=f"px{qc}_{kb}_{h}")
                            if causal and diag:
                                nc.vector.tensor_add(pl[:, q0:q0 + P], pl[:, q0:q0 + P],
                                                     maskDg_t[:])
                                if q0 > 0:
                                    nc.vector.memset(px[:, :q0], 0.0)
                            elif not causal:
                                nc.vector.tensor_add(pl[:], pl[:], mtt[:])
                            nc.scalar.activation(px[:, q0:], pl[:, q0:], AF.Exp)
                            if kb == 0:
                                nc.gpsimd.tensor_copy(dens[h][:], px[:])
                            else:
                                nc.gpsimd.tensor_add(dens[h][:, q0:], dens[h][:, q0:],
                                                     px[:, q0:])
                            pxs.append(px)
                        for h in range(NHC):
                            nc.tensor.matmul(avs[h][:], lhsT=V[kb][:, h * VD:(h + 1) * VD],
                                             rhs=pxs[h][:], start=(kb == 0),
                                             stop=(kb == kb_hi - 1))
                    # project next chunk's Q while softmax denominators finish
                    if qc + 1 < NQC:
                        qnext = qproj(qc + 1, qldnext)
                    o_t = []
                    rcps = []
                    for h in range(NHC):
                        dps = psl.tile([1, SC], F32, tag="pl", name=f"dps{qc}_{h}")
                        nc.tensor.matmul(dps[:], lhsT=ones_t[:], rhs=dens[h][:],
                                         start=True, stop=True)
                        dsb = attpool.tile([1, SC], F32, tag="dsb", name=f"dsb{qc}_{h}")
                        nc.scalar.copy(dsb[:], dps[:])
                        rcp = rcppool.tile([1, SC], FR, tag=f"rcp{h}", name=f"rcp{qc}_{h}")
                        with nc.allow_low_precision(reason="f32r recip for prob norm"):
                            nc.vector.reciprocal(rcp[:], dsb[:])
                        rcps.append(rcp)
                    for h in range(NHC):
                        rbp = psl.tile([VD, SC], F32, tag="pl", name=f"rbp{qc}_{h}")
                        nc.tensor.matmul(rbp[:], lhsT=ones_row[:], rhs=rcps[h][:],
                                         start=True, stop=True)
                        rbb = attpool.tile([VD, SC], F32, tag="rbb", name=f"rbb{qc}_{h}")
                        nc.scalar.copy(rbb[:], rbp[:])
                        o = opool.tile([VD, SC], BF, tag=f"o{h}", name=f"o{h}_{qc}")
                        nc.vector.tensor_mul(o[:], avs[h][:], rbb[:])
                        o_t.append(o)
                    for ho in range(H // P):
                        ps = psq.tile([P, SC], F32, tag="pq", name=f"po{qc}_{ho}")
                        for j in range(NKVB):
                            nc.tensor.matmul(ps[:], lhsT=Wo_t[:, j, ho * P:(ho + 1) * P],
                                             rhs=o_t[j][:], start=(j == 0),
                                             stop=(j == NKVB - 1))
                        ot = oopool.tile([P, SC], F32, tag="ot", name=f"ot{qc}_{ho}")
                        nc.scalar.copy(ot[:], ps[:])
                        nc.sync.dma_start(out=outT[ho * P:(ho + 1) * P, qsl], in_=ot[:])
                    if qc + 1 < NQC:
                        qcur = qnext

    if not os.environ.get("KSIM"):
        split_multiwaits(nc)
    return nc


def _pack_front(Wfull):
    """[4096, 2112 kv-first] -> [128, 17*32*128] bf16, zero-padded rope block."""
    out = np.zeros((P, N_FB * N_KI, P), np.float32)
    off = 0
    for fb, w in enumerate(FB_W):
        blk = Wfull[:, off:off + w].reshape(N_KI, P, w).transpose(1, 0, 2)
        out[:, fb * N_KI:(fb + 1) * N_KI, :w] = blk
        off += w
    return np.ascontiguousarray(out.reshape(P, -1)).astype(ml_dtypes.bfloat16)


def _pack_k(WT, nhw):
    """[K, nhw] -> [128, (K//128)*nhw]: k-tile-major packing of a T-layout weight."""
    K = WT.shape[0]
    t = WT.reshape(K // P, P, nhw).transpose(1, 0, 2).reshape(P, (K // P) * nhw)
    return np.ascontiguousarray(t).astype(ml_dtypes.bfloat16)


def _rope_tables():
    inv = 1.0 / (BASE ** (np.arange(0, ROPE, 2, dtype=np.float64) / ROPE))
    t = np.arange(S, dtype=np.float64)
    fr_ = np.outer(t, inv)
    emb = np.concatenate([fr_, fr_], axis=1)
    cos = np.cos(emb).T.astype(np.float32)   # [64, S]
    sin = np.sin(emb).T.astype(np.float32)
    ssin = sin.copy()
    ssin[:32] *= -1.0
    return cos, ssin


def kernel(hidden_states, attention_mask, Wqa, qa_ln_w, Wqb, Wkva, kva_ln_w, Wkvb, Wo):
    hidden_states = np.asarray(hidden_states, np.float32)
    attention_mask = np.asarray(attention_mask, np.float32)
    Wqa = np.asarray(Wqa, np.float32)
    Wqb = np.asarray(Wqb, np.float32)
    Wkva = np.asarray(Wkva, np.float32)
    Wkvb = np.asarray(Wkvb, np.float32)
    Wo = np.asarray(Wo, np.float32)
    qa_ln_w = np.asarray(qa_ln_w, np.float32)
    kva_ln_w = np.asarray(kva_ln_w, np.float32)

    mask = attention_mask[0, 0]
    tril = np.tril(np.ones((S, S), bool))
    causal = bool(np.array_equal(mask, np.where(tril, 0.0, -1e9).astype(np.float32)))

    hT = np.ascontiguousarray(hidden_states[0].T)            # [H, S]
    maskT = np.ascontiguousarray(mask.T)
    cos, ssin = _rope_tables()
    cq2_full = np.concatenate([cos, cos], axis=0)            # [128, S]
    sq2_full = np.concatenate([ssin, ssin], axis=0)

    # single diagonal mask band tile: maskDg[k, q] = 0 if k <= q else -1e9
    kk = np.arange(P)[:, None]
    qq = np.arange(P)[None, :]
    maskDg = np.where(kk <= qq, 0.0, -1e9).astype(np.float32).astype(ml_dtypes.bfloat16)

    # front weights, kv-first: [kvn 512 | rope 64 | q_a 1536] columns of [H x .]
    WkvaT = np.ascontiguousarray(Wkva.T)                     # [H, 576]
    WqaT = np.ascontiguousarray(Wqa.T)                       # [H, 1536]
    Wfull = np.concatenate([WkvaT[:, :KVLR], WkvaT[:, KVLR:], WqaT], axis=1)
    Wp = _pack_front(Wfull)

    Wqb_eff = (Wqb * qa_ln_w[None, :]).astype(np.float32) * np.float32(SCALE)
    Wkvb_eff = (Wkvb * kva_ln_w[None, :]).astype(np.float32)

    in_maps = []
    for c in range(NCORES):
        csl = slice(c * SLC, (c + 1) * SLC)
        # hidden slice, k-tile-major packed [128, 32*256]
        hs_slice = hT[:, csl].reshape(N_KI, P, SLC).transpose(1, 0, 2)
        hTs_p = np.ascontiguousarray(hs_slice.reshape(P, -1)).astype(ml_dtypes.bfloat16)
        # Wqb head-slice, columns reordered: 4x nope then 2x packed rope pairs
        hsl = slice(c * NHC * QHD, (c + 1) * NHC * QHD)
        Wq = Wqb_eff[hsl]                                    # [768, 1536]
        rows = []
        for h in range(NHC):
            rows.append(Wq[h * QHD:h * QHD + NOPE])          # nope rows
        for pr in range(2):
            for hh in range(2):
                h = 2 * pr + hh
                rows.append(Wq[h * QHD + NOPE:(h + 1) * QHD])  # rope rows (64)
        Wq_re = np.concatenate(rows, axis=0)                 # [768, 1536]
        # Wkvb head-slice: 4x K rows then 4x V rows
        ksl = slice(c * NHC * (NOPE + VD), (c + 1) * NHC * (NOPE + VD))
        Wkv = Wkvb_eff[ksl]                                  # [1024, 512]
        rows = [Wkv[h * (NOPE + VD):h * (NOPE + VD) + NOPE] for h in range(NHC)]
        rows += [Wkv[h * (NOPE + VD) + NOPE:(h + 1) * (NOPE + VD)] for h in range(NHC)]
        Wkv_re = np.concatenate(rows, axis=0)                # [1024, 512]
        osl = slice(c * NHC * VD, (c + 1) * NHC * VD)
        in_maps.append({
            "hTs_p": hTs_p,
            "Wp": Wp,
            "Wqb_p": _pack_k(np.ascontiguousarray(Wq_re.T), NHC * QHD),
            "Wkvb_p": _pack_k(np.ascontiguousarray(Wkv_re.T), NHC * (NOPE + VD)),
            "Wo_p": _pack_k(np.ascontiguousarray(Wo[:, osl].T), H),
            "cq2": cq2_full, "sq2": sq2_full,
            "cqk": np.ascontiguousarray(cos[:, csl]),
            "sqk": np.ascontiguousarray(ssin[:, csl]),
            "maskDg": maskDg, "maskT": maskT,
        })

    nc = build(causal)
    trace = bool(os.environ.get("KPROF"))
    kw = {}
    td = os.environ.get("KPROF_DIR")
    if trace and td:
        os.makedirs(td, exist_ok=True)
        kw["tmpdir"] = td
    res = run_bass_kernel_spmd(nc, in_maps, list(range(NCORES)), trace=trace, **kw)
    if trace:
        print(f"HW exec time: {res.exec_time_ns} ns (mean {res.mean_exec_time_ns}, "
              f"max core {res.max_exec_time_core_id})")
    acc = res.results[0]["outT"].astype(np.float64)
    for c in range(1, NCORES):
        acc += res.results[c]["outT"]
    return np.ascontiguousarray(acc.T)[None, :, :].astype(np.float32)
